# revision 1
# baseline (speedup 1.0000x reference)
"""Trainium2 Bass kernel for nn_CumulativeFlattenedLinear (segment_reduce).

Computation: per window of S=64 timesteps, per-timestep C->O linear projection
(weights zero for the first n_discard steps) followed by a causal cumsum within
the window, plus bias.

Strategy (data-parallel over batch, 1 batch element per core):
  - Reformulate per 8-step sub-block u: a triangular-masked "intra" matmul plus
    a "prefix" matmul whose target axis is the later sub-blocks; both share a
    transposed-x stationary and are issued as ONE stacked N=256 fp32r matmul
    writing [intra | pre] contiguously in PSUM (pre region shared per u-pair,
    accumulated in PSUM).
  - x is loaded with partition = 256-element time chunk (1KB contiguous DMA
    runs), shuffled on-chip to (u, c, v') column order (GPSIMD), transposed
    128x128 on the TensorEngine, rounded to fp32r during the batched
    PSUM->SBUF copies (ScalarE).
  - prefix totals summed across the 3 pair-regions + bias (DVE), then one
    strided combine per window writes the (o, t)-ordered output tile, stored
    with 1KB contiguous runs.
"""
import numpy as np

import concourse.bass as bass
import concourse.tile as tile
from concourse import bacc, mybir
from concourse.bass_utils import run_bass_kernel_spmd

F32 = mybir.dt.float32
F32R = mybir.dt.float32r

# problem geometry (asserted against inputs at runtime)
B, C, T, O = 8, 16, 131072, 16
P = 128
CH = 256                 # time-elements per partition per supertile
NST = T // (P * CH)      # 4 supertiles
V = 8                    # sub-block length
NU = 8                   # sub-blocks per window

_cache = {}


def _build_nc(du_count, mm_dtype=F32R):
    """Build the per-core Bass program. du_count = number of active sub-blocks
    (those with any nonzero weight), assumed to be the trailing ones."""
    S = NU * V  # 64
    NW = CH // S  # windows per partition = 4
    DU = du_count
    first_u = NU - DU          # first active sub-block
    fill_s = first_u * V       # s < fill_s -> output = bias

    nc = bacc.Bacc("TRN2", target_bir_lowering=False, debug=False)
    x_d = nc.dram_tensor("x", (C, T), F32, kind="ExternalInput")
    w_d = nc.dram_tensor("w_all", (P, DU * 256), mm_dtype, kind="ExternalInput")
    bpre_d = nc.dram_tensor("bias_pre", (P, P), F32, kind="ExternalInput")
    ident_d = nc.dram_tensor("ident", (P, P), F32, kind="ExternalInput")
    bfill_d = nc.dram_tensor("bias_fill", (P, O * fill_s), F32,
                             kind="ExternalInput")
    y_d = nc.dram_tensor("y", (O, T), F32, kind="ExternalOutput")

    xv = x_d.ap().rearrange("c (st p hs) -> st p c hs", st=NST, p=P, hs=CH)
    yv = y_d.ap().rearrange("o (st p hs) -> st p o hs", st=NST, p=P, hs=CH)

    NB = (DU + 1) // 2  # psum banks per window group

    with tile.TileContext(nc) as tc:
        with (
            tc.tile_pool(name="const", bufs=1) as cp,
            tc.tile_pool(name="io", bufs=2) as io,
            tc.tile_pool(name="mid", bufs=3) as mid,
            tc.tile_pool(name="psW", bufs=2, space="PSUM") as psW,
            tc.tile_pool(name="psT", bufs=2, space="PSUM") as psT,
        ):
            w_all = cp.tile([P, DU * 256], mm_dtype, name="w_all")
            nc.sync.dma_start(w_all[:], w_d.ap())
            bias_pre = cp.tile([P, P], F32, name="bias_pre")
            nc.sync.dma_start(bias_pre[:], bpre_d.ap())
            ident = cp.tile([P, P], F32, name="ident")
            nc.sync.dma_start(ident[:], ident_d.ap())
            bfill = cp.tile([P, O * fill_s], F32, name="bfill")
            nc.sync.dma_start(bfill[:], bfill_d.ap())

            for st in range(NST):
                xin = io.tile([P, C * CH], F32, name="xin", tag="xin")
                nc.sync.dma_start(
                    xin[:].rearrange("p (c hs) -> p c hs", c=C), xv[st]
                )
                out_sb = io.tile([P, O * CH], F32, name="out_sb", tag="out")
                for wdw in range(NW):
                    # ---- shuffle to (du, c, v') ----
                    shuf = mid.tile([P, DU * 128], F32, name="shuf", tag="shuf")
                    src = xin[:].rearrange(
                        "p (c w u v) -> w p u c v", c=C, w=NW, u=NU, v=V
                    )[wdw, :, first_u:NU]
                    nc.vector.tensor_copy(
                        shuf[:].rearrange("p (u c v) -> p u c v", u=DU, c=C, v=V),
                        src,
                    )
                    # ---- transposes (PE) in groups of <=4 per psum bank ----
                    tsb = []
                    du = 0
                    gi = 0
                    while du < DU:
                        n = min(4, DU - du)
                        pt = psT.tile([P, 512], F32, name=f"pt{gi}", tag="pt")
                        for j in range(n):
                            nc.tensor.transpose(
                                pt[:, j * 128:(j + 1) * 128],
                                shuf[:, (du + j) * 128:(du + j + 1) * 128],
                                ident[:],
                                tile_position=(0, 0),
                            )
                        ts = mid.tile([P, n * 128], mm_dtype,
                                      name=f"ts{gi}", tag=f"ts{gi}")
                        nc.scalar.copy(ts[:], pt[:, 0:n * 128])
                        for j in range(n):
                            tsb.append((ts, j))
                        du += n
                        gi += 1
                    # ---- matmuls ----
                    pw = psW.tile([P, NB * 512], F32, name="pw", tag="pw")
                    for du in range(DU):
                        bk = du // 2
                        lo = bk * 512 + (du % 2) * 128
                        ts, j = tsb[du]
                        nc.tensor.matmul(
                            pw[:, lo:lo + 256],
                            ts[:, j * 128:(j + 1) * 128],
                            w_all[:, du * 256:(du + 1) * 256],
                            start=(du % 2 == 0),
                            stop=(du % 2 == 1 or du == DU - 1),
                            skip_group_check=True,
                        )
                    # ---- prefix totals: pre_s = sum of pre regions ----
                    pre_s = mid.tile([P, P], F32, name="pre_s", tag="pre_s")
                    nc.vector.tensor_add(pre_s[:], bias_pre[:], pw[:, 128:256])
                    for bk in range(1, NB):
                        nc.vector.tensor_add(
                            pre_s[:], pre_s[:],
                            pw[:, bk * 512 + 128:bk * 512 + 256],
                        )
                    # ---- combine: out[(o, s)] = intra + pre_bcast ----
                    # out col = o*CH + wdw*S + s,  s = (first_u + du)*V + v
                    out4 = out_sb[:].rearrange(
                        "p (o w u v) -> w p o u v", o=O, w=NW, u=NU, v=V
                    )[wdw, :, :, first_u:NU]
                    # in1: psum intra: col = bk*512 + (du%2)*256 + v*16 + o
                    in1 = pw[:].rearrange(
                        "p (bk half x) -> p bk half x", bk=NB, half=2
                    )[:, :, :, 0:128]
                    in1 = in1.rearrange(
                        "p bk half (v o) -> p o (bk half) v", v=V, o=O
                    )
                    # in2: pre_s col = (first_u + du)*16 + o, step0 over v
                    in2 = pre_s[:].rearrange("p (u o) -> p u o", u=NU)
                    in2 = in2[:, first_u:NU]
                    in2 = in2.transpose([0, 2, 1]).unsqueeze(3)
                    in2 = in2.broadcast_to([P, O, DU, V])
                    nc.vector.tensor_add(out4, in1, in2)
                    # ---- bias fill for s < fill_s (ACT) ----
                    outf = out_sb[:].rearrange(
                        "p (o w s) -> w p o s", o=O, w=NW
                    )[wdw, :, :, 0:fill_s]
                    nc.scalar.copy(
                        outf,
                        bfill[:].rearrange("p (o s) -> p o s", o=O),
                    )
                nc.scalar.dma_start(
                    yv[st], out_sb[:].rearrange("p (o hs) -> p o hs", o=O)
                )
    nc.compile()
    return nc


def _host_constants(weight, bias, n_discard, n_keep, mm_np=np.float32):
    S = n_discard + n_keep
    assert S == NU * V
    w = weight.reshape(O, C, n_keep).transpose(2, 1, 0)  # (n_keep, C, O)
    w_full = np.concatenate(
        [np.zeros((n_discard, C, O), np.float32), w.astype(np.float32)], axis=0
    )  # (S, C, O)
    act = [u for u in range(NU)
           if np.abs(w_full[u * V:(u + 1) * V]).max() > 0]
    # kernel assumes active blocks are trailing & contiguous
    first_u = act[0] if act else NU
    assert act == list(range(first_u, NU))
    DU = len(act)
    rhs = np.zeros((DU, P, 256), np.float32)
    vp_idx = np.arange(V)
    for idx, u in enumerate(act):
        blk = w_full[u * V:(u + 1) * V]  # (V, C, O)
        # Wtri: k=(c,vp) -> n=(v,o)
        tri = np.zeros((C, V, V, O), np.float32)
        for v in range(V):
            tri[:, vp_idx <= v, v, :] = blk.transpose(1, 0, 2)[:, vp_idx <= v]
        Wtri = tri.reshape(C * V, V * O)
        # Wpre: k=(c,vp) -> n=(ut,o)
        pre = np.zeros((C, V, NU, O), np.float32)
        for ut in range(NU):
            if ut > u:
                pre[:, :, ut, :] = blk.transpose(1, 0, 2)
        Wpre = pre.reshape(C * V, NU * O)
        if idx % 2 == 0:
            rhs[idx] = np.concatenate([Wtri, Wpre], axis=1)
        else:
            rhs[idx] = np.concatenate([Wpre, Wtri], axis=1)
    w_all = rhs.transpose(1, 0, 2).reshape(P, DU * 256).astype(mm_np)
    bias32 = bias.astype(np.float32)
    consts = {
        "w_all": np.ascontiguousarray(w_all),
        "bias_pre": np.ascontiguousarray(
            np.tile(bias32, NU)[None, :] * np.ones((P, 1), np.float32)
        ),
        "ident": np.eye(P, dtype=np.float32),
        "bias_fill": np.ascontiguousarray(
            np.tile(bias32[:, None], (1, first_u * V)).reshape(1, -1)
            * np.ones((P, 1), np.float32)
        ),
    }
    return consts, DU


def _run(inputs, trace=False):
    x = np.asarray(inputs["x"], dtype=np.float32)
    weight = np.asarray(inputs["weight"], dtype=np.float32)
    bias = np.asarray(inputs["bias"], dtype=np.float32)
    n_discard = int(inputs["n_discard"])
    n_keep = int(inputs["n_keep"])
    assert x.shape == (B, C, T) and weight.shape == (O, C * n_keep)

    consts, DU = _host_constants(weight, bias, n_discard, n_keep)
    key = ("nc", DU)
    if key not in _cache:
        _cache[key] = _build_nc(DU)
    nc = _cache[key]

    in_maps = []
    for b in range(B):
        m = dict(consts)
        m["x"] = np.ascontiguousarray(x[b])
        in_maps.append(m)
    res = run_bass_kernel_spmd(nc, in_maps, list(range(B)), trace=trace)
    y = np.stack([res.results[b]["y"] for b in range(B)], axis=0)
    return y, res


def kernel(**inputs):
    y, _ = _run(inputs, trace=False)
    return y



# revision 5
# speedup vs baseline: 1.0456x; 1.0456x over previous
"""Trainium2 Bass kernel for nn_CumulativeFlattenedLinear (segment_reduce).

Computation: per window of S=64 timesteps, per-timestep C->O linear projection
(weights zero for the first n_discard steps) followed by a causal cumsum within
the window, plus bias.

Strategy (data-parallel over batch, 1 batch element per core), v2:
  - x is DMA'd with partition = 256-element time chunk (1KB contiguous runs).
  - One DVE copy per supertile fuses the (c,w,u,v)->(w,u,c,v) column shuffle
    with an fp32->fp16 cast (2x DVE mode).
  - Per window: 6 PE transposes (fp16) of the sub-block columns into ONE fp16
    PSUM bank; a single ACT copy moves them to SBUF as the matmul stationary.
  - Per sub-block u: a triangular "intra" matmul (n=128) plus a thin "pre"
    matmul (n<=96) that writes the sub-block's totals into the slot-suffix of a
    shared PSUM region; PSUM accumulation across sub-blocks yields the per-slot
    prefix sums directly (no DVE tree adds).
  - DVE adds bias to the prefix region, then one strided combine per window
    writes the (o, t)-ordered output tile; GPSIMD fills the bias-only prefix
    timesteps; output stored with 1KB contiguous runs.
"""
import numpy as np

import concourse.bass as bass
import concourse.tile as tile
from concourse import bacc, mybir
from concourse.bass_utils import run_bass_kernel_spmd

F32 = mybir.dt.float32
F16 = mybir.dt.float16

# problem geometry (asserted against inputs at runtime)
B, C, T, O = 8, 16, 131072, 16
P = 128
CH = 256                 # time-elements per partition per supertile
NST = T // (P * CH)      # 4 supertiles
V = 8                    # sub-block length
NU = 8                   # sub-blocks per window
S = NU * V               # 64
NW = CH // S             # windows per partition = 4

_cache = {}


def _pre_slices(DU):
    """Per-du (psum_off, ncols) for the thin pre matmuls. Slot du' in the
    shared [P, DU*O] pre region accumulates totals of sub-blocks du < du'.
    du=0 writes the whole region (slot 0 zero-weighted) so start=True covers
    it; later du write their slot-suffix."""
    out = []
    for du in range(DU - 1):
        if du == 0:
            out.append((0, DU * O))
        else:
            out.append(((du + 1) * O, (DU - 1 - du) * O))
    return out


def _build_nc(du_count):
    DU = du_count
    assert DU == 6, "start/stop flag layout below assumes DU=6"
    first_u = NU - DU          # first active sub-block
    fill_s = first_u * V       # s < fill_s -> output = bias
    pre_sl = _pre_slices(DU)
    PRE_W = sum(n for _, n in pre_sl)
    # PSUM bank layout for the [P, DU*128 + DU*O] f32 tile (bank = 512 f32):
    #   bank0: tri du=0..3; bank1: tri du=4..5 + pre region.
    # One accumulation group per bank: start=True only on the first writer,
    # stop=True on the last. Program order in bank1: pre0, pre1..4
    # interleaved with tri4, tri5 (tri writes land on fresh columns).
    tri_start = {0: True, 1: False, 2: False, 3: False, 4: False, 5: False}
    tri_stop = {0: False, 1: False, 2: False, 3: True, 4: False, 5: True}

    nc = bacc.Bacc("TRN2", target_bir_lowering=False, debug=False)
    x_d = nc.dram_tensor("x", (C, T), F32, kind="ExternalInput")
    wtri_d = nc.dram_tensor("w_tri", (P, DU * 128), F16, kind="ExternalInput")
    wpre_d = nc.dram_tensor("w_pre", (P, PRE_W), F16, kind="ExternalInput")
    bpre_d = nc.dram_tensor("bias_pre", (P, DU * O), F32, kind="ExternalInput")
    ident_d = nc.dram_tensor("ident", (P, P), F16, kind="ExternalInput")
    bfill_d = nc.dram_tensor("bias_fill", (P, O * fill_s), F32,
                             kind="ExternalInput")
    y_d = nc.dram_tensor("y", (O, T), F32, kind="ExternalOutput")

    xv = x_d.ap().rearrange("c (st p hs) -> st p c hs", st=NST, p=P, hs=CH)
    yv = y_d.ap().rearrange("o (st p hs) -> st p o hs", st=NST, p=P, hs=CH)

    with tile.TileContext(nc) as tc:
        with (
            tc.tile_pool(name="const", bufs=1) as cp,
            tc.tile_pool(name="io", bufs=2) as io,
            tc.tile_pool(name="mid", bufs=3) as mid,
            tc.tile_pool(name="psT", bufs=2, space="PSUM") as psT,
            tc.tile_pool(name="psW", bufs=2, space="PSUM") as psW,
        ):
            wtri = cp.tile([P, DU * 128], F16, name="w_tri")
            nc.sync.dma_start(wtri[:], wtri_d.ap())
            wpre = cp.tile([P, PRE_W], F16, name="w_pre")
            nc.sync.dma_start(wpre[:], wpre_d.ap())
            bias_pre = cp.tile([P, DU * O], F32, name="bias_pre")
            nc.sync.dma_start(bias_pre[:], bpre_d.ap())
            ident = cp.tile([P, P], F16, name="ident")
            nc.sync.dma_start(ident[:], ident_d.ap())
            bfill = cp.tile([P, O * fill_s], F32, name="bfill")
            nc.sync.dma_start(bfill[:], bfill_d.ap())

            # The bias-only prefix timesteps (s < fill_s) sit at fixed column
            # offsets of the out tiles and are never overwritten by the
            # combine; write them once per round-robin buffer up front and let
            # every supertile's output DMA re-read them.
            for _ in range(2):
                ob = io.tile([P, O * CH], F32, name="out_sb", tag="out")
                dst = ob[:].rearrange(
                    "p (o w s) -> p o w s", o=O, w=NW
                )[:, :, :, 0:fill_s]
                src = bfill[:].rearrange("p (o s) -> p o s", o=O)
                src = src.unsqueeze(2).broadcast_to([P, O, NW, fill_s])
                nc.scalar.copy(dst, src)

            for st in range(NST):
                xin = io.tile([P, C * CH], F32, name="xin", tag="xin")
                nc.sync.dma_start(
                    xin[:].rearrange("p (c hs) -> p c hs", c=C), xv[st]
                )
                # fused shuffle + fp16 cast for the whole supertile (GPSIMD)
                shuf = mid.tile([P, NW * DU * 128], F16, name="shuf",
                                tag="shuf")
                src = xin[:].rearrange(
                    "p (c w u v) -> p w u c v", c=C, w=NW, u=NU, v=V
                )[:, :, first_u:NU]
                nc.gpsimd.tensor_copy(
                    shuf[:].rearrange(
                        "p (w u c v) -> p w u c v", w=NW, u=DU, c=C, v=V
                    ),
                    src,
                )
                out_sb = io.tile([P, O * CH], F32, name="out_sb", tag="out")
                for wdw in range(NW):
                    # ---- transposes (PE) into one fp16 psum bank ----
                    ptw = psT.tile([P, DU * 128], F16, name="ptw", tag="ptw")
                    for du in range(DU):
                        nc.tensor.transpose(
                            ptw[:, du * 128:(du + 1) * 128],
                            shuf[:, (wdw * DU + du) * 128:
                                 (wdw * DU + du + 1) * 128],
                            ident[:],
                            tile_position=(0, 0),
                        )
                    ts = mid.tile([P, DU * 128], F16, name="ts", tag="ts")
                    nc.scalar.copy(ts[:], ptw[:])
                    # ---- matmuls: tri (n=128) + thin pre (slot suffix) ----
                    pw = psW.tile([P, DU * 128 + DU * O], F32, name="pw",
                                  tag="pw")
                    woff = 0
                    for du in range(DU):
                        lhs = ts[:, du * 128:(du + 1) * 128]
                        if du < DU - 1:
                            # pre before tri so bank1's group opens with pre0
                            off, n = pre_sl[du]
                            nc.tensor.matmul(
                                pw[:, DU * 128 + off:DU * 128 + off + n],
                                lhs,
                                wpre[:, woff:woff + n],
                                start=(du == 0), stop=False,
                                skip_group_check=True,
                            )
                            woff += n
                        nc.tensor.matmul(
                            pw[:, du * 128:(du + 1) * 128],
                            lhs,
                            wtri[:, du * 128:(du + 1) * 128],
                            start=tri_start[du], stop=tri_stop[du],
                            skip_group_check=True,
                        )
                    # ---- prefix totals + bias (DVE) ----
                    pre_sb = mid.tile([P, DU * O], F32, name="pre_sb",
                                      tag="pre_sb")
                    nc.vector.tensor_add(
                        pre_sb[:], pw[:, DU * 128:DU * 128 + DU * O],
                        bias_pre[:],
                    )
                    # ---- combine: out[(o, s)] = intra + pre_bcast ----
                    out4 = out_sb[:].rearrange(
                        "p (o w u v) -> w p o u v", o=O, w=NW, u=NU, v=V
                    )[wdw, :, :, first_u:NU]
                    in1 = pw[:, 0:DU * 128].rearrange(
                        "p (du v o) -> p o du v", du=DU, v=V, o=O
                    )
                    in2 = pre_sb[:].rearrange("p (du o) -> p du o", du=DU)
                    in2 = in2.transpose([0, 2, 1]).unsqueeze(3)
                    in2 = in2.broadcast_to([P, O, DU, V])
                    nc.vector.tensor_add(out4, in1, in2)
                    # ---- bias fill for s < fill_s (GPSIMD) ----
                    outf = out_sb[:].rearrange(
                        "p (o w s) -> w p o s", o=O, w=NW
                    )[wdw, :, :, 0:fill_s]
                    nc.gpsimd.tensor_copy(
                        outf,
                        bfill[:].rearrange("p (o s) -> p o s", o=O),
                    )
                nc.scalar.dma_start(
                    yv[st], out_sb[:].rearrange("p (o hs) -> p o hs", o=O)
                )
    nc.compile()
    return nc


def _host_constants(weight, bias, n_discard, n_keep):
    assert n_discard + n_keep == S
    w = weight.reshape(O, C, n_keep).transpose(2, 1, 0)  # (n_keep, C, O)
    w_full = np.concatenate(
        [np.zeros((n_discard, C, O), np.float32), w.astype(np.float32)], axis=0
    )  # (S, C, O)
    act = [u for u in range(NU)
           if np.abs(w_full[u * V:(u + 1) * V]).max() > 0]
    first_u = act[0] if act else NU
    assert act == list(range(first_u, NU))
    DU = len(act)
    vp_idx = np.arange(V)
    tri_blocks = []
    blk_tot = []  # per-du (C*V, O) total-weights
    for u in act:
        blk = w_full[u * V:(u + 1) * V]  # (V, C, O)
        tri = np.zeros((C, V, V, O), np.float32)
        for v in range(V):
            tri[:, vp_idx <= v, v, :] = blk.transpose(1, 0, 2)[:, vp_idx <= v]
        tri_blocks.append(tri.reshape(C * V, V * O))
        blk_tot.append(blk.transpose(1, 0, 2).reshape(C * V, O))
    w_tri = np.concatenate(tri_blocks, axis=1)  # (128, DU*128)
    # thin pre blocks: du writes slots (du', o); slot du' accumulates totals
    # of earlier sub-blocks
    pre_cols = []
    for du, (off, n) in enumerate(_pre_slices(DU)):
        nslots = n // O
        lo_slot = off // O
        blkw = np.zeros((C * V, nslots, O), np.float32)
        for j in range(nslots):
            if lo_slot + j > du:  # slot index du' > du gets tot_du
                blkw[:, j, :] = blk_tot[du]
        pre_cols.append(blkw.reshape(C * V, n))
    w_pre = (np.concatenate(pre_cols, axis=1) if pre_cols
             else np.zeros((C * V, 0), np.float32))
    bias32 = bias.astype(np.float32)
    fill_s = first_u * V
    consts = {
        "w_tri": np.ascontiguousarray(w_tri.astype(np.float16)),
        "w_pre": np.ascontiguousarray(w_pre.astype(np.float16)),
        "bias_pre": np.ascontiguousarray(
            np.tile(bias32, DU)[None, :] * np.ones((P, 1), np.float32)
        ),
        "ident": np.ascontiguousarray(np.eye(P, dtype=np.float16)),
        "bias_fill": np.ascontiguousarray(
            np.tile(bias32[:, None], (1, fill_s)).reshape(1, -1)
            * np.ones((P, 1), np.float32)
        ),
    }
    return consts, DU


def _run(inputs, trace=False):
    x = np.asarray(inputs["x"], dtype=np.float32)
    weight = np.asarray(inputs["weight"], dtype=np.float32)
    bias = np.asarray(inputs["bias"], dtype=np.float32)
    n_discard = int(inputs["n_discard"])
    n_keep = int(inputs["n_keep"])
    assert x.shape == (B, C, T) and weight.shape == (O, C * n_keep)

    consts, DU = _host_constants(weight, bias, n_discard, n_keep)
    key = ("nc", DU)
    if key not in _cache:
        _cache[key] = _build_nc(DU)
    nc = _cache[key]

    in_maps = []
    for b in range(B):
        m = dict(consts)
        m["x"] = np.ascontiguousarray(x[b])
        in_maps.append(m)
    res = run_bass_kernel_spmd(nc, in_maps, list(range(B)), trace=trace)
    y = np.stack([res.results[b]["y"] for b in range(B)], axis=0)
    return y, res


def kernel(**inputs):
    y, _ = _run(inputs, trace=False)
    return y


# revision 7
# speedup vs baseline: 1.3257x; 1.2679x over previous
"""Trainium2 Bass kernel for nn_CumulativeFlattenedLinear (segment_reduce).

Computation: per window of S=64 timesteps, per-timestep C->O linear projection
(weights zero for the first n_discard steps) followed by a causal cumsum within
the window, plus bias.

Strategy (data-parallel over batch, 1 batch element per core), v2:
  - x is DMA'd with partition = 256-element time chunk (1KB contiguous runs).
  - One DVE copy per supertile fuses the (c,w,u,v)->(w,u,c,v) column shuffle
    with an fp32->fp16 cast (2x DVE mode).
  - Per window: 6 PE transposes (fp16) of the sub-block columns into ONE fp16
    PSUM bank; a single ACT copy moves them to SBUF as the matmul stationary.
  - Per sub-block u: a triangular "intra" matmul (n=128) plus a thin "pre"
    matmul (n<=96) that writes the sub-block's totals into the slot-suffix of a
    shared PSUM region; PSUM accumulation across sub-blocks yields the per-slot
    prefix sums directly (no DVE tree adds).
  - DVE adds bias to the prefix region, then one strided combine per window
    writes the (o, t)-ordered output tile; GPSIMD fills the bias-only prefix
    timesteps; output stored with 1KB contiguous runs.
"""
import numpy as np

import concourse.bass as bass
import concourse.tile as tile
from concourse import bacc, mybir
from concourse.bass_utils import run_bass_kernel_spmd

F32 = mybir.dt.float32
F16 = mybir.dt.float16

# problem geometry (asserted against inputs at runtime)
B, C, T, O = 8, 16, 131072, 16
P = 128
CH = 256                 # time-elements per partition per supertile
NST = T // (P * CH)      # 4 supertiles
V = 8                    # sub-block length
NU = 8                   # sub-blocks per window
S = NU * V               # 64
NW = CH // S             # windows per partition = 4

_cache = {}


def _pre_slices(DU):
    """Per-du (psum_off, ncols) for the thin pre matmuls. Slot du' in the
    shared [P, DU*O] pre region accumulates totals of sub-blocks du < du'.
    du=0 writes the whole region (slot 0 zero-weighted) so start=True covers
    it; later du write their slot-suffix."""
    out = []
    for du in range(DU - 1):
        if du == 0:
            out.append((0, DU * O))
        else:
            out.append(((du + 1) * O, (DU - 1 - du) * O))
    return out


def _build_nc(du_count):
    DU = du_count
    assert DU == 6, "start/stop flag layout below assumes DU=6"
    first_u = NU - DU          # first active sub-block
    fill_s = first_u * V       # s < fill_s -> output = bias
    pre_sl = _pre_slices(DU)
    PRE_W = sum(n for _, n in pre_sl)
    # PSUM bank layout for the [P, DU*128 + DU*O] f32 tile (bank = 512 f32):
    #   bank0: tri du=0..3; bank1: tri du=4..5 + pre region.
    # One accumulation group per bank: start=True only on the first writer,
    # stop=True on the last. Program order in bank1: pre0, pre1..4
    # interleaved with tri4, tri5 (tri writes land on fresh columns).
    tri_start = {0: True, 1: False, 2: False, 3: False, 4: False, 5: False}
    tri_stop = {0: False, 1: False, 2: False, 3: True, 4: False, 5: True}

    nc = bacc.Bacc("TRN2", target_bir_lowering=False, debug=False)
    x_d = nc.dram_tensor("x", (C, T), F32, kind="ExternalInput")
    wtri_d = nc.dram_tensor("w_tri", (P, DU * 128), F16, kind="ExternalInput")
    wpre_d = nc.dram_tensor("w_pre", (P, PRE_W), F16, kind="ExternalInput")
    bpre_d = nc.dram_tensor("bias_pre", (P, DU * O), F32, kind="ExternalInput")
    ident_d = nc.dram_tensor("ident", (P, P), F16, kind="ExternalInput")
    bfill_d = nc.dram_tensor("bias_fill", (P, O * fill_s), F32,
                             kind="ExternalInput")
    y_d = nc.dram_tensor("y", (O, T), F32, kind="ExternalOutput")

    xv = x_d.ap().rearrange("c (st p hs) -> st p c hs", st=NST, p=P, hs=CH)
    yv = y_d.ap().rearrange("o (st p hs) -> st p o hs", st=NST, p=P, hs=CH)

    with tile.TileContext(nc) as tc:
        with (
            tc.tile_pool(name="const", bufs=1) as cp,
            tc.tile_pool(name="io", bufs=2) as io,
            tc.tile_pool(name="mid", bufs=3) as mid,
            tc.tile_pool(name="psT", bufs=2, space="PSUM") as psT,
            tc.tile_pool(name="psW", bufs=2, space="PSUM") as psW,
        ):
            wtri = cp.tile([P, DU * 128], F16, name="w_tri")
            nc.sync.dma_start(wtri[:], wtri_d.ap())
            wpre = cp.tile([P, PRE_W], F16, name="w_pre")
            nc.sync.dma_start(wpre[:], wpre_d.ap())
            bias_pre = cp.tile([P, DU * O], F32, name="bias_pre")
            nc.sync.dma_start(bias_pre[:], bpre_d.ap())
            ident = cp.tile([P, P], F16, name="ident")
            nc.sync.dma_start(ident[:], ident_d.ap())
            bfill = cp.tile([P, O * fill_s], F32, name="bfill")
            nc.sync.dma_start(bfill[:], bfill_d.ap())

            # The bias-only prefix timesteps (s < fill_s) sit at fixed column
            # offsets of the out tiles and are never overwritten by the
            # combine; write them once per round-robin buffer up front and let
            # every supertile's output DMA re-read them.
            for _ in range(2):
                ob = io.tile([P, O * CH], F32, name="out_sb", tag="out")
                dst = ob[:].rearrange(
                    "p (o w s) -> p o w s", o=O, w=NW
                )[:, :, :, 0:fill_s]
                src = bfill[:].rearrange("p (o s) -> p o s", o=O)
                src = src.unsqueeze(2).broadcast_to([P, O, NW, fill_s])
                nc.scalar.copy(dst, src)

            for st in range(NST):
                xin = io.tile([P, C * CH], F32, name="xin", tag="xin")
                nc.sync.dma_start(
                    xin[:].rearrange("p (c hs) -> p c hs", c=C), xv[st]
                )
                # fused shuffle + fp16 cast for the whole supertile (DVE 2x)
                shuf = mid.tile([P, NW * DU * 128], F16, name="shuf",
                                tag="shuf")
                src = xin[:].rearrange(
                    "p (c w u v) -> p w u c v", c=C, w=NW, u=NU, v=V
                )[:, :, first_u:NU]
                nc.vector.tensor_copy(
                    shuf[:].rearrange(
                        "p (w u c v) -> p w u c v", w=NW, u=DU, c=C, v=V
                    ),
                    src,
                )
                out_sb = io.tile([P, O * CH], F32, name="out_sb", tag="out")
                for wdw in range(NW):
                    # ---- transposes (PE) into one fp16 psum bank ----
                    ptw = psT.tile([P, DU * 128], F16, name="ptw", tag="ptw")
                    for du in range(DU):
                        nc.tensor.transpose(
                            ptw[:, du * 128:(du + 1) * 128],
                            shuf[:, (wdw * DU + du) * 128:
                                 (wdw * DU + du + 1) * 128],
                            ident[:],
                            tile_position=(0, 0),
                        )
                    ts = mid.tile([P, DU * 128], F16, name="ts", tag="ts")
                    nc.scalar.copy(ts[:], ptw[:])
                    # ---- matmuls: tri (n=128) + thin pre (slot suffix) ----
                    pw = psW.tile([P, DU * 128 + DU * O], F32, name="pw",
                                  tag="pw")
                    woff = 0
                    for du in range(DU):
                        lhs = ts[:, du * 128:(du + 1) * 128]
                        if du < DU - 1:
                            # pre before tri so bank1's group opens with pre0
                            off, n = pre_sl[du]
                            nc.tensor.matmul(
                                pw[:, DU * 128 + off:DU * 128 + off + n],
                                lhs,
                                wpre[:, woff:woff + n],
                                start=(du == 0), stop=False,
                                skip_group_check=True,
                            )
                            woff += n
                        nc.tensor.matmul(
                            pw[:, du * 128:(du + 1) * 128],
                            lhs,
                            wtri[:, du * 128:(du + 1) * 128],
                            start=tri_start[du], stop=tri_stop[du],
                            skip_group_check=True,
                        )
                    # ---- prefix totals + bias (DVE) ----
                    pre_sb = mid.tile([P, DU * O], F32, name="pre_sb",
                                      tag="pre_sb")
                    nc.vector.tensor_add(
                        pre_sb[:], pw[:, DU * 128:DU * 128 + DU * O],
                        bias_pre[:],
                    )
                    # ---- combine: out[(o, s)] = intra + pre_bcast ----
                    out4 = out_sb[:].rearrange(
                        "p (o w u v) -> w p o u v", o=O, w=NW, u=NU, v=V
                    )[wdw, :, :, first_u:NU]
                    in1 = pw[:, 0:DU * 128].rearrange(
                        "p (du v o) -> p o du v", du=DU, v=V, o=O
                    )
                    in2 = pre_sb[:].rearrange("p (du o) -> p du o", du=DU)
                    in2 = in2.transpose([0, 2, 1]).unsqueeze(3)
                    in2 = in2.broadcast_to([P, O, DU, V])
                    nc.vector.tensor_add(out4, in1, in2)
                nc.scalar.dma_start(
                    yv[st], out_sb[:].rearrange("p (o hs) -> p o hs", o=O)
                )
    nc.compile()
    return nc


def _host_constants(weight, bias, n_discard, n_keep):
    assert n_discard + n_keep == S
    w = weight.reshape(O, C, n_keep).transpose(2, 1, 0)  # (n_keep, C, O)
    w_full = np.concatenate(
        [np.zeros((n_discard, C, O), np.float32), w.astype(np.float32)], axis=0
    )  # (S, C, O)
    act = [u for u in range(NU)
           if np.abs(w_full[u * V:(u + 1) * V]).max() > 0]
    first_u = act[0] if act else NU
    assert act == list(range(first_u, NU))
    DU = len(act)
    vp_idx = np.arange(V)
    tri_blocks = []
    blk_tot = []  # per-du (C*V, O) total-weights
    for u in act:
        blk = w_full[u * V:(u + 1) * V]  # (V, C, O)
        tri = np.zeros((C, V, V, O), np.float32)
        for v in range(V):
            tri[:, vp_idx <= v, v, :] = blk.transpose(1, 0, 2)[:, vp_idx <= v]
        tri_blocks.append(tri.reshape(C * V, V * O))
        blk_tot.append(blk.transpose(1, 0, 2).reshape(C * V, O))
    w_tri = np.concatenate(tri_blocks, axis=1)  # (128, DU*128)
    # thin pre blocks: du writes slots (du', o); slot du' accumulates totals
    # of earlier sub-blocks
    pre_cols = []
    for du, (off, n) in enumerate(_pre_slices(DU)):
        nslots = n // O
        lo_slot = off // O
        blkw = np.zeros((C * V, nslots, O), np.float32)
        for j in range(nslots):
            if lo_slot + j > du:  # slot index du' > du gets tot_du
                blkw[:, j, :] = blk_tot[du]
        pre_cols.append(blkw.reshape(C * V, n))
    w_pre = (np.concatenate(pre_cols, axis=1) if pre_cols
             else np.zeros((C * V, 0), np.float32))
    bias32 = bias.astype(np.float32)
    fill_s = first_u * V
    consts = {
        "w_tri": np.ascontiguousarray(w_tri.astype(np.float16)),
        "w_pre": np.ascontiguousarray(w_pre.astype(np.float16)),
        "bias_pre": np.ascontiguousarray(
            np.tile(bias32, DU)[None, :] * np.ones((P, 1), np.float32)
        ),
        "ident": np.ascontiguousarray(np.eye(P, dtype=np.float16)),
        "bias_fill": np.ascontiguousarray(
            np.tile(bias32[:, None], (1, fill_s)).reshape(1, -1)
            * np.ones((P, 1), np.float32)
        ),
    }
    return consts, DU


def _run(inputs, trace=False):
    x = np.asarray(inputs["x"], dtype=np.float32)
    weight = np.asarray(inputs["weight"], dtype=np.float32)
    bias = np.asarray(inputs["bias"], dtype=np.float32)
    n_discard = int(inputs["n_discard"])
    n_keep = int(inputs["n_keep"])
    assert x.shape == (B, C, T) and weight.shape == (O, C * n_keep)

    consts, DU = _host_constants(weight, bias, n_discard, n_keep)
    key = ("nc", DU)
    if key not in _cache:
        _cache[key] = _build_nc(DU)
    nc = _cache[key]

    in_maps = []
    for b in range(B):
        m = dict(consts)
        m["x"] = np.ascontiguousarray(x[b])
        in_maps.append(m)
    res = run_bass_kernel_spmd(nc, in_maps, list(range(B)), trace=trace)
    y = np.stack([res.results[b]["y"] for b in range(B)], axis=0)
    return y, res


def kernel(**inputs):
    y, _ = _run(inputs, trace=False)
    return y


# revision 9
# speedup vs baseline: 1.3774x; 1.0390x over previous
"""Trainium2 Bass kernel for nn_CumulativeFlattenedLinear (segment_reduce).

Computation: per window of S=64 timesteps, per-timestep C->O linear projection
(weights zero for the first n_discard steps) followed by a causal cumsum within
the window, plus bias.

Strategy (data-parallel over batch, 1 batch element per core), v2:
  - x is DMA'd with partition = 256-element time chunk (1KB contiguous runs).
  - One DVE copy per supertile fuses the (c,w,u,v)->(w,u,c,v) column shuffle
    with an fp32->fp16 cast (2x DVE mode).
  - Per window: 6 PE transposes (fp16) of the sub-block columns into ONE fp16
    PSUM bank; a single ACT copy moves them to SBUF as the matmul stationary.
  - Per sub-block u: a triangular "intra" matmul (n=128) plus a thin "pre"
    matmul (n<=96) that writes the sub-block's totals into the slot-suffix of a
    shared PSUM region; PSUM accumulation across sub-blocks yields the per-slot
    prefix sums directly (no DVE tree adds).
  - DVE adds bias to the prefix region, then one strided combine per window
    writes the (o, t)-ordered output tile; GPSIMD fills the bias-only prefix
    timesteps; output stored with 1KB contiguous runs.
"""
import numpy as np

import concourse.bass as bass
import concourse.tile as tile
from concourse import bacc, mybir
from concourse.bass_utils import run_bass_kernel_spmd

F32 = mybir.dt.float32
F16 = mybir.dt.float16

# problem geometry (asserted against inputs at runtime)
B, C, T, O = 8, 16, 131072, 16
P = 128
CH = 256                 # time-elements per partition per supertile
NST = T // (P * CH)      # 4 supertiles
V = 8                    # sub-block length
NU = 8                   # sub-blocks per window
S = NU * V               # 64
NW = CH // S             # windows per partition = 4

_cache = {}


def _pre_slices(DU):
    """Per-du (psum_off, ncols) for the thin pre matmuls. Slot du' in the
    shared [P, DU*O] pre region accumulates totals of sub-blocks du < du'.
    du=0 writes the whole region (slot 0 zero-weighted) so start=True covers
    it; later du write their slot-suffix."""
    out = []
    for du in range(DU - 1):
        if du == 0:
            out.append((0, DU * O))
        else:
            out.append(((du + 1) * O, (DU - 1 - du) * O))
    return out


def _build_nc(du_count):
    DU = du_count
    assert DU == 6, "start/stop flag layout below assumes DU=6"
    first_u = NU - DU          # first active sub-block
    fill_s = first_u * V       # s < fill_s -> output = bias
    pre_sl = _pre_slices(DU)
    PRE_W = sum(n for _, n in pre_sl)
    # PSUM bank layout for the [P, DU*128 + DU*O] f32 tile (bank = 512 f32):
    #   bank0: tri du=0..3; bank1: tri du=4..5 + pre region.
    # One accumulation group per bank: start=True only on the first writer,
    # stop=True on the last. Program order in bank1: pre0, pre1..4
    # interleaved with tri4, tri5 (tri writes land on fresh columns).
    tri_start = {0: True, 1: False, 2: False, 3: False, 4: False, 5: False}
    tri_stop = {0: False, 1: False, 2: False, 3: True, 4: False, 5: True}

    nc = bacc.Bacc("TRN2", target_bir_lowering=False, debug=False)
    x_d = nc.dram_tensor("x", (C, T), F32, kind="ExternalInput")
    wtri_d = nc.dram_tensor("w_tri", (P, DU * 128), F16, kind="ExternalInput")
    wpre_d = nc.dram_tensor("w_pre", (P, PRE_W), F16, kind="ExternalInput")
    bpre_d = nc.dram_tensor("bias_pre", (P, DU * O), F32, kind="ExternalInput")
    ident_d = nc.dram_tensor("ident", (P, P), F16, kind="ExternalInput")
    bfill_d = nc.dram_tensor("bias_fill", (P, O * fill_s), F32,
                             kind="ExternalInput")
    y_d = nc.dram_tensor("y", (O, T), F32, kind="ExternalOutput")

    xv = x_d.ap().rearrange("c (st p hs) -> st p c hs", st=NST, p=P, hs=CH)
    yv = y_d.ap().rearrange("o (st p hs) -> st p o hs", st=NST, p=P, hs=CH)

    with tile.TileContext(nc) as tc:
        with (
            tc.tile_pool(name="const", bufs=1) as cp,
            tc.tile_pool(name="io", bufs=2) as io,
            tc.tile_pool(name="mid", bufs=3) as mid,
            tc.tile_pool(name="psT", bufs=2, space="PSUM") as psT,
            tc.tile_pool(name="psW", bufs=2, space="PSUM") as psW,
        ):
            # All input DMAs go first on the sync HWDGE ring; alternating tags
            # (2 bufs each -> 4 resident buffers) keep each trigger free of
            # write-after-read waits on earlier shuffles, so the input stream
            # runs back-to-back at line rate.
            xins = []
            for st in range(NST):
                xin = io.tile([P, C * CH], F32, name=f"xin{st}",
                              tag=f"xin{st % 2}")
                nc.sync.dma_start(
                    xin[:].rearrange("p (c hs) -> p c hs", c=C), xv[st]
                )
                xins.append(xin)

            # constants ride the scalar HWDGE ring (idle until the first
            # output DMA) so they don't delay the input stream
            wtri = cp.tile([P, DU * 128], F16, name="w_tri")
            nc.scalar.dma_start(wtri[:], wtri_d.ap())
            wpre = cp.tile([P, PRE_W], F16, name="w_pre")
            nc.scalar.dma_start(wpre[:], wpre_d.ap())
            bias_pre = cp.tile([P, DU * O], F32, name="bias_pre")
            nc.scalar.dma_start(bias_pre[:], bpre_d.ap())
            ident = cp.tile([P, P], F16, name="ident")
            nc.scalar.dma_start(ident[:], ident_d.ap())
            bfill = cp.tile([P, O * fill_s], F32, name="bfill")
            nc.scalar.dma_start(bfill[:], bfill_d.ap())

            # The bias-only prefix timesteps (s < fill_s) sit at fixed column
            # offsets of the out tiles and are never overwritten by the
            # combine; write them once per round-robin buffer up front and let
            # every supertile's output DMA re-read them.
            for _ in range(2):
                ob = io.tile([P, O * CH], F32, name="out_sb", tag="out")
                dst = ob[:].rearrange(
                    "p (o w s) -> p o w s", o=O, w=NW
                )[:, :, :, 0:fill_s]
                src = bfill[:].rearrange("p (o s) -> p o s", o=O)
                src = src.unsqueeze(2).broadcast_to([P, O, NW, fill_s])
                nc.scalar.copy(dst, src)

            for st in range(NST):
                xin = xins[st]
                # fused shuffle + fp16 cast for the whole supertile (DVE 2x)
                shuf = mid.tile([P, NW * DU * 128], F16, name="shuf",
                                tag="shuf")
                src = xin[:].rearrange(
                    "p (c w u v) -> p w u c v", c=C, w=NW, u=NU, v=V
                )[:, :, first_u:NU]
                nc.vector.tensor_copy(
                    shuf[:].rearrange(
                        "p (w u c v) -> p w u c v", w=NW, u=DU, c=C, v=V
                    ),
                    src,
                )
                out_sb = io.tile([P, O * CH], F32, name="out_sb", tag="out")
                for wdw in range(NW):
                    # ---- transposes (PE) into one fp16 psum bank ----
                    ptw = psT.tile([P, DU * 128], F16, name="ptw", tag="ptw")
                    for du in range(DU):
                        nc.tensor.transpose(
                            ptw[:, du * 128:(du + 1) * 128],
                            shuf[:, (wdw * DU + du) * 128:
                                 (wdw * DU + du + 1) * 128],
                            ident[:],
                            tile_position=(0, 0),
                        )
                    ts = mid.tile([P, DU * 128], F16, name="ts", tag="ts")
                    nc.scalar.copy(ts[:], ptw[:])
                    # ---- matmuls: tri (n=128) + thin pre (slot suffix) ----
                    pw = psW.tile([P, DU * 128 + DU * O], F32, name="pw",
                                  tag="pw")
                    woff = 0
                    for du in range(DU):
                        lhs = ts[:, du * 128:(du + 1) * 128]
                        if du < DU - 1:
                            # pre before tri so bank1's group opens with pre0
                            off, n = pre_sl[du]
                            nc.tensor.matmul(
                                pw[:, DU * 128 + off:DU * 128 + off + n],
                                lhs,
                                wpre[:, woff:woff + n],
                                start=(du == 0), stop=False,
                                skip_group_check=True,
                            )
                            woff += n
                        nc.tensor.matmul(
                            pw[:, du * 128:(du + 1) * 128],
                            lhs,
                            wtri[:, du * 128:(du + 1) * 128],
                            start=tri_start[du], stop=tri_stop[du],
                            skip_group_check=True,
                        )
                    # ---- prefix totals + bias (DVE) ----
                    pre_sb = mid.tile([P, DU * O], F32, name="pre_sb",
                                      tag="pre_sb")
                    nc.vector.tensor_add(
                        pre_sb[:], pw[:, DU * 128:DU * 128 + DU * O],
                        bias_pre[:],
                    )
                    # ---- combine: out[(o, s)] = intra + pre_bcast ----
                    out4 = out_sb[:].rearrange(
                        "p (o w u v) -> w p o u v", o=O, w=NW, u=NU, v=V
                    )[wdw, :, :, first_u:NU]
                    in1 = pw[:, 0:DU * 128].rearrange(
                        "p (du v o) -> p o du v", du=DU, v=V, o=O
                    )
                    in2 = pre_sb[:].rearrange("p (du o) -> p du o", du=DU)
                    in2 = in2.transpose([0, 2, 1]).unsqueeze(3)
                    in2 = in2.broadcast_to([P, O, DU, V])
                    nc.vector.tensor_add(out4, in1, in2)
                nc.scalar.dma_start(
                    yv[st], out_sb[:].rearrange("p (o hs) -> p o hs", o=O)
                )
    nc.compile()
    return nc


def _host_constants(weight, bias, n_discard, n_keep):
    assert n_discard + n_keep == S
    w = weight.reshape(O, C, n_keep).transpose(2, 1, 0)  # (n_keep, C, O)
    w_full = np.concatenate(
        [np.zeros((n_discard, C, O), np.float32), w.astype(np.float32)], axis=0
    )  # (S, C, O)
    act = [u for u in range(NU)
           if np.abs(w_full[u * V:(u + 1) * V]).max() > 0]
    first_u = act[0] if act else NU
    assert act == list(range(first_u, NU))
    DU = len(act)
    vp_idx = np.arange(V)
    tri_blocks = []
    blk_tot = []  # per-du (C*V, O) total-weights
    for u in act:
        blk = w_full[u * V:(u + 1) * V]  # (V, C, O)
        tri = np.zeros((C, V, V, O), np.float32)
        for v in range(V):
            tri[:, vp_idx <= v, v, :] = blk.transpose(1, 0, 2)[:, vp_idx <= v]
        tri_blocks.append(tri.reshape(C * V, V * O))
        blk_tot.append(blk.transpose(1, 0, 2).reshape(C * V, O))
    w_tri = np.concatenate(tri_blocks, axis=1)  # (128, DU*128)
    # thin pre blocks: du writes slots (du', o); slot du' accumulates totals
    # of earlier sub-blocks
    pre_cols = []
    for du, (off, n) in enumerate(_pre_slices(DU)):
        nslots = n // O
        lo_slot = off // O
        blkw = np.zeros((C * V, nslots, O), np.float32)
        for j in range(nslots):
            if lo_slot + j > du:  # slot index du' > du gets tot_du
                blkw[:, j, :] = blk_tot[du]
        pre_cols.append(blkw.reshape(C * V, n))
    w_pre = (np.concatenate(pre_cols, axis=1) if pre_cols
             else np.zeros((C * V, 0), np.float32))
    bias32 = bias.astype(np.float32)
    fill_s = first_u * V
    consts = {
        "w_tri": np.ascontiguousarray(w_tri.astype(np.float16)),
        "w_pre": np.ascontiguousarray(w_pre.astype(np.float16)),
        "bias_pre": np.ascontiguousarray(
            np.tile(bias32, DU)[None, :] * np.ones((P, 1), np.float32)
        ),
        "ident": np.ascontiguousarray(np.eye(P, dtype=np.float16)),
        "bias_fill": np.ascontiguousarray(
            np.tile(bias32[:, None], (1, fill_s)).reshape(1, -1)
            * np.ones((P, 1), np.float32)
        ),
    }
    return consts, DU


def _run(inputs, trace=False):
    x = np.asarray(inputs["x"], dtype=np.float32)
    weight = np.asarray(inputs["weight"], dtype=np.float32)
    bias = np.asarray(inputs["bias"], dtype=np.float32)
    n_discard = int(inputs["n_discard"])
    n_keep = int(inputs["n_keep"])
    assert x.shape == (B, C, T) and weight.shape == (O, C * n_keep)

    consts, DU = _host_constants(weight, bias, n_discard, n_keep)
    key = ("nc", DU)
    if key not in _cache:
        _cache[key] = _build_nc(DU)
    nc = _cache[key]

    in_maps = []
    for b in range(B):
        m = dict(consts)
        m["x"] = np.ascontiguousarray(x[b])
        in_maps.append(m)
    res = run_bass_kernel_spmd(nc, in_maps, list(range(B)), trace=trace)
    y = np.stack([res.results[b]["y"] for b in range(B)], axis=0)
    return y, res


def kernel(**inputs):
    y, _ = _run(inputs, trace=False)
    return y


# revision 12
# speedup vs baseline: 1.3825x; 1.0037x over previous
"""Trainium2 Bass kernel for nn_CumulativeFlattenedLinear (segment_reduce).

Computation: per window of S=64 timesteps, per-timestep C->O linear projection
(weights zero for the first n_discard steps) followed by a causal cumsum within
the window, plus bias.

Strategy (data-parallel over batch, 1 batch element per core), v2:
  - x is DMA'd with partition = 256-element time chunk (1KB contiguous runs).
  - One DVE copy per supertile fuses the (c,w,u,v)->(w,u,c,v) column shuffle
    with an fp32->fp16 cast (2x DVE mode).
  - Per window: 6 PE transposes (fp16) of the sub-block columns into ONE fp16
    PSUM bank; a single ACT copy moves them to SBUF as the matmul stationary.
  - Per sub-block u: a triangular "intra" matmul (n=128) plus a thin "pre"
    matmul (n<=96) that writes the sub-block's totals into the slot-suffix of a
    shared PSUM region; PSUM accumulation across sub-blocks yields the per-slot
    prefix sums directly (no DVE tree adds).
  - DVE adds bias to the prefix region, then one strided combine per window
    writes the (o, t)-ordered output tile; GPSIMD fills the bias-only prefix
    timesteps; output stored with 1KB contiguous runs.
"""
import numpy as np

import concourse.bass as bass
import concourse.tile as tile
from concourse import bacc, mybir
from concourse.bass_utils import run_bass_kernel_spmd

F32 = mybir.dt.float32
F16 = mybir.dt.float16

# problem geometry (asserted against inputs at runtime)
B, C, T, O = 8, 16, 131072, 16
P = 128
CH = 256                 # time-elements per partition per supertile
NST = T // (P * CH)      # 4 supertiles
V = 8                    # sub-block length
NU = 8                   # sub-blocks per window
S = NU * V               # 64
NW = CH // S             # windows per partition = 4

_cache = {}


def _pre_slices(DU):
    """Per-du (psum_off, ncols) for the thin pre matmuls. Slot du' in the
    shared [P, DU*O] pre region accumulates totals of sub-blocks du < du'.
    du=0 writes the whole region (slot 0 zero-weighted) so start=True covers
    it; later du write their slot-suffix."""
    out = []
    for du in range(DU - 1):
        if du == 0:
            out.append((0, DU * O))
        else:
            out.append(((du + 1) * O, (DU - 1 - du) * O))
    return out


def _build_nc(du_count):
    DU = du_count
    assert DU == 6, "start/stop flag layout below assumes DU=6"
    first_u = NU - DU          # first active sub-block
    fill_s = first_u * V       # s < fill_s -> output = bias
    pre_sl = _pre_slices(DU)
    PRE_W = sum(n for _, n in pre_sl)
    # PSUM bank layout for the [P, DU*128 + DU*O] f32 tile (bank = 512 f32):
    #   bank0: tri du=0..3; bank1: tri du=4..5 + pre region.
    # One accumulation group per bank: start=True only on the first writer,
    # stop=True on the last. Program order in bank1: pre0, pre1..4
    # interleaved with tri4, tri5 (tri writes land on fresh columns).
    tri_start = {0: True, 1: False, 2: False, 3: False, 4: False, 5: False}
    tri_stop = {0: False, 1: False, 2: False, 3: True, 4: False, 5: True}

    nc = bacc.Bacc("TRN2", target_bir_lowering=False, debug=False)
    x_d = nc.dram_tensor("x", (C, T), F32, kind="ExternalInput")
    wtri_d = nc.dram_tensor("w_tri", (P, DU * 128), F16, kind="ExternalInput")
    wpre_d = nc.dram_tensor("w_pre", (P, PRE_W), F16, kind="ExternalInput")
    bpre_d = nc.dram_tensor("bias_pre", (P, DU * O), F32, kind="ExternalInput")
    ident_d = nc.dram_tensor("ident", (P, P), F16, kind="ExternalInput")
    bfill_d = nc.dram_tensor("bias_fill", (P, O * fill_s), F32,
                             kind="ExternalInput")
    y_d = nc.dram_tensor("y", (O, T), F32, kind="ExternalOutput")

    xv = x_d.ap().rearrange("c (st p hs) -> st p c hs", st=NST, p=P, hs=CH)
    yv = y_d.ap().rearrange("o (st p hs) -> st p o hs", st=NST, p=P, hs=CH)

    with tile.TileContext(nc) as tc:
        with (
            tc.tile_pool(name="const", bufs=1) as cp,
            tc.tile_pool(name="io", bufs=2) as io,
            tc.tile_pool(name="mid", bufs=3) as mid,
            tc.tile_pool(name="psT", bufs=2, space="PSUM") as psT,
            tc.tile_pool(name="psW", bufs=3, space="PSUM") as psW,
        ):
            # All input DMAs go first on the sync HWDGE ring; alternating tags
            # (2 bufs each -> 4 resident buffers) keep each trigger free of
            # write-after-read waits on earlier shuffles, so the input stream
            # runs back-to-back at line rate.
            xins = []
            for st in range(NST):
                xin = io.tile([P, C * CH], F32, name=f"xin{st}",
                              tag=f"xin{st % 2}")
                nc.sync.dma_start(
                    xin[:].rearrange("p (c hs) -> p c hs", c=C), xv[st]
                )
                xins.append(xin)

            # constants ride the scalar HWDGE ring (idle until the first
            # output DMA) so they don't delay the input stream
            wtri = cp.tile([P, DU * 128], F16, name="w_tri")
            nc.scalar.dma_start(wtri[:], wtri_d.ap())
            wpre = cp.tile([P, PRE_W], F16, name="w_pre")
            nc.scalar.dma_start(wpre[:], wpre_d.ap())
            bias_pre = cp.tile([P, DU * O], F32, name="bias_pre")
            nc.scalar.dma_start(bias_pre[:], bpre_d.ap())
            ident = cp.tile([P, P], F16, name="ident")
            nc.scalar.dma_start(ident[:], ident_d.ap())
            bfill = cp.tile([P, O * fill_s], F32, name="bfill")
            nc.scalar.dma_start(bfill[:], bfill_d.ap())

            # The bias-only prefix timesteps (s < fill_s) sit at fixed column
            # offsets of the out tiles and are never overwritten by the
            # combine; write them once per round-robin buffer up front and let
            # every supertile's output DMA re-read them.
            for _ in range(2):
                ob = io.tile([P, O * CH], F32, name="out_sb", tag="out")
                dst = ob[:].rearrange(
                    "p (o w s) -> p o w s", o=O, w=NW
                )[:, :, :, 0:fill_s]
                src = bfill[:].rearrange("p (o s) -> p o s", o=O)
                src = src.unsqueeze(2).broadcast_to([P, O, NW, fill_s])
                nc.scalar.copy(dst, src)

            for st in range(NST):
                xin = xins[st]
                # fused shuffle + fp16 cast for the whole supertile (DVE 2x)
                shuf = mid.tile([P, NW * DU * 128], F16, name="shuf",
                                tag="shuf")
                src = xin[:].rearrange(
                    "p (c w u v) -> p w u c v", c=C, w=NW, u=NU, v=V
                )[:, :, first_u:NU]
                nc.vector.tensor_copy(
                    shuf[:].rearrange(
                        "p (w u c v) -> p w u c v", w=NW, u=DU, c=C, v=V
                    ),
                    src,
                )
                out_sb = io.tile([P, O * CH], F32, name="out_sb", tag="out")
                for wdw in range(NW):
                    # ---- transposes (PE) into one fp16 psum bank ----
                    ptw = psT.tile([P, DU * 128], F16, name="ptw", tag="ptw")
                    for du in range(DU):
                        nc.tensor.transpose(
                            ptw[:, du * 128:(du + 1) * 128],
                            shuf[:, (wdw * DU + du) * 128:
                                 (wdw * DU + du + 1) * 128],
                            ident[:],
                            tile_position=(0, 0),
                        )
                    ts = mid.tile([P, DU * 128], F16, name="ts", tag="ts")
                    nc.scalar.copy(ts[:], ptw[:])
                    # ---- matmuls: tri (n=128) + thin pre (slot suffix) ----
                    pw = psW.tile([P, DU * 128 + DU * O], F32, name="pw",
                                  tag="pw")
                    woff = 0
                    for du in range(DU):
                        lhs = ts[:, du * 128:(du + 1) * 128]
                        if du < DU - 1:
                            # pre before tri so bank1's group opens with pre0
                            off, n = pre_sl[du]
                            nc.tensor.matmul(
                                pw[:, DU * 128 + off:DU * 128 + off + n],
                                lhs,
                                wpre[:, woff:woff + n],
                                start=(du == 0), stop=False,
                                skip_group_check=True,
                            )
                            woff += n
                        nc.tensor.matmul(
                            pw[:, du * 128:(du + 1) * 128],
                            lhs,
                            wtri[:, du * 128:(du + 1) * 128],
                            start=tri_start[du], stop=tri_stop[du],
                            skip_group_check=True,
                        )
                    # ---- prefix totals + bias (DVE) ----
                    pre_sb = mid.tile([P, DU * O], F32, name="pre_sb",
                                      tag="pre_sb")
                    nc.vector.tensor_add(
                        pre_sb[:], pw[:, DU * 128:DU * 128 + DU * O],
                        bias_pre[:],
                    )
                    # ---- combine: out[(o, s)] = intra + pre_bcast ----
                    out4 = out_sb[:].rearrange(
                        "p (o w u v) -> w p o u v", o=O, w=NW, u=NU, v=V
                    )[wdw, :, :, first_u:NU]
                    in1 = pw[:, 0:DU * 128].rearrange(
                        "p (du v o) -> p o du v", du=DU, v=V, o=O
                    )
                    in2 = pre_sb[:].rearrange("p (du o) -> p du o", du=DU)
                    in2 = in2.transpose([0, 2, 1]).unsqueeze(3)
                    in2 = in2.broadcast_to([P, O, DU, V])
                    nc.vector.tensor_add(out4, in1, in2)
                nc.scalar.dma_start(
                    yv[st], out_sb[:].rearrange("p (o hs) -> p o hs", o=O)
                )
    nc.compile()
    return nc


def _host_constants(weight, bias, n_discard, n_keep):
    assert n_discard + n_keep == S
    w = weight.reshape(O, C, n_keep).transpose(2, 1, 0)  # (n_keep, C, O)
    w_full = np.concatenate(
        [np.zeros((n_discard, C, O), np.float32), w.astype(np.float32)], axis=0
    )  # (S, C, O)
    act = [u for u in range(NU)
           if np.abs(w_full[u * V:(u + 1) * V]).max() > 0]
    first_u = act[0] if act else NU
    assert act == list(range(first_u, NU))
    DU = len(act)
    vp_idx = np.arange(V)
    tri_blocks = []
    blk_tot = []  # per-du (C*V, O) total-weights
    for u in act:
        blk = w_full[u * V:(u + 1) * V]  # (V, C, O)
        tri = np.zeros((C, V, V, O), np.float32)
        for v in range(V):
            tri[:, vp_idx <= v, v, :] = blk.transpose(1, 0, 2)[:, vp_idx <= v]
        tri_blocks.append(tri.reshape(C * V, V * O))
        blk_tot.append(blk.transpose(1, 0, 2).reshape(C * V, O))
    w_tri = np.concatenate(tri_blocks, axis=1)  # (128, DU*128)
    # thin pre blocks: du writes slots (du', o); slot du' accumulates totals
    # of earlier sub-blocks
    pre_cols = []
    for du, (off, n) in enumerate(_pre_slices(DU)):
        nslots = n // O
        lo_slot = off // O
        blkw = np.zeros((C * V, nslots, O), np.float32)
        for j in range(nslots):
            if lo_slot + j > du:  # slot index du' > du gets tot_du
                blkw[:, j, :] = blk_tot[du]
        pre_cols.append(blkw.reshape(C * V, n))
    w_pre = (np.concatenate(pre_cols, axis=1) if pre_cols
             else np.zeros((C * V, 0), np.float32))
    bias32 = bias.astype(np.float32)
    fill_s = first_u * V
    consts = {
        "w_tri": np.ascontiguousarray(w_tri.astype(np.float16)),
        "w_pre": np.ascontiguousarray(w_pre.astype(np.float16)),
        "bias_pre": np.ascontiguousarray(
            np.tile(bias32, DU)[None, :] * np.ones((P, 1), np.float32)
        ),
        "ident": np.ascontiguousarray(np.eye(P, dtype=np.float16)),
        "bias_fill": np.ascontiguousarray(
            np.tile(bias32[:, None], (1, fill_s)).reshape(1, -1)
            * np.ones((P, 1), np.float32)
        ),
    }
    return consts, DU


def _run(inputs, trace=False):
    x = np.asarray(inputs["x"], dtype=np.float32)
    weight = np.asarray(inputs["weight"], dtype=np.float32)
    bias = np.asarray(inputs["bias"], dtype=np.float32)
    n_discard = int(inputs["n_discard"])
    n_keep = int(inputs["n_keep"])
    assert x.shape == (B, C, T) and weight.shape == (O, C * n_keep)

    consts, DU = _host_constants(weight, bias, n_discard, n_keep)
    key = ("nc", DU)
    if key not in _cache:
        _cache[key] = _build_nc(DU)
    nc = _cache[key]

    in_maps = []
    for b in range(B):
        m = dict(consts)
        m["x"] = np.ascontiguousarray(x[b])
        in_maps.append(m)
    res = run_bass_kernel_spmd(nc, in_maps, list(range(B)), trace=trace)
    y = np.stack([res.results[b]["y"] for b in range(B)], axis=0)
    return y, res


def kernel(**inputs):
    y, _ = _run(inputs, trace=False)
    return y
